# revision 28
# baseline (speedup 1.0000x reference)
"""Trainium2 Bass/Tile kernel for the GatedNode2Edge op.

Computes, for emb (B,C,N), th12_* (E,C), th5_* (E,):
    t_k  = th12_k @ emb[b]                      (E,N)
    m_k  = max(t_k[:,i], t_k[:,j]) pairwise     (E,N,N)
    adj  = relu(2*m_1 + th5_1*I)
    gate = sigmoid(relu(2*m_2 + th5_2*I))
    out  = adj * gate                           (B,E,N,N)

Sharding: the 64 (b,e) channels are split 8-per-core across 8 NeuronCores.

Math restructuring (off-diagonal):
    relu(2*max(a,b)) = max(2*relu(a), 2*relu(b))           (relu monotone)
    sigmoid(max(x,y)) = max(sigmoid(x), sigmoid(y))        (sigmoid monotone)
    sigmoid(2*relu(t)) = max(sigmoid(2t), 1/2)             (relu fold)
so with row vectors v = 2*relu(t1), g = sigmoid(2*t2):
    out[i,j] = max(v_i, v_j) * max(g_i, g_j, 1/2)
one fused custom-DVE op per [128, N] output tile:
    out = maxx(Src0, C0) * maxx(Src1, C1)
with Src0 = v broadcast across partitions (PE outer-product), C0 = v column
slice (per-partition scalar), Src1 = sigmoid(2t2) broadcast, and C1 =
max(g_i, 1/2) column slice (the relu fold rides in the scalar).

Performance structure vs the f32 baseline:
  - All pairwise tiles are fp16; custom DVE op runs a hand-authored
    2X_1PORT uop program (2 outputs/cycle); fp16 matmuls (single pass).
  - Replication PSUM is allocated in 1-bank halves so it coexists with
    the phase-1 PSUM: channel 0/1 replication overlaps the phase-1 tail
    instead of stalling on a PSUM-bank WAR against phase 2.
  - Emission order hand-interleaved so the first custom-DVE op fires as
    early as possible; ACT tables are pre-warmed by dummy activations.
  - True diagonal patched with ONE strided copy_predicated per channel;
    output leaves per channel as one 2 MB DMA (last channel split in two
    to shorten the tail).
"""

import sys
import types

import numpy as np

B, C, N, E = 2, 64, 1024, 32
NCORES = 8
EPC = B * E // NCORES  # 8 channels per core
P = 128
NB = N // P  # 8 row blocks
H = N // 2  # matmul moving free-dim limit is 512

PERF2X = True  # set False to fall back to 1x custom-DVE mode

_CACHE = {}


def _ensure_hook_shim():
    """Make trace=True safe even when antenv.axon_hooks is absent."""
    try:
        import antenv.axon_hooks  # noqa: F401
    except ImportError:
        mod = types.ModuleType("antenv.axon_hooks")
        mod.get_axon_ntff_profile_hook = lambda: None
        mod.set_axon_ntff_profile_hook = lambda h: None
        sys.modules["antenv.axon_hooks"] = mod


def _build_2x_uop():
    """2X_1PORT program for out = max(Src0,C0)*max(max(Src1,C1),C2).

    Per cycle the engine delivers packed fp16 pairs: (SRC_0, SRC_0_HI) from
    rd0 and (SRC_1, SRC_1_HI) from rd1.  All eight ALU blocks compute (four
    ops per result, two results per cycle); the even (lo) result rides delay
    chain 1 to the writeback, the odd (hi) result leaves via block-7 ALU out.
    """
    from concourse.dve_uop import (
        AluInp,
        AluOp,
        DelayInp,
        InpSel,
        OutPath,
        OutSel,
        Trigger,
        UopConfig,
    )

    u = UopConfig()
    # lane 0 feeds block 0's ALU directly; lanes 1..6 feed delay chains 0..5
    u.enable_input(InpSel.SRC_0, 0)      # direct ALU input at b0
    u.enable_input(InpSel.CONST_0, 1)    # chain 0
    u.enable_input(InpSel.SRC_1, 2)      # chain 1
    u.enable_input(InpSel.CONST_1, 3)    # chain 2
    u.enable_input(InpSel.SRC_0_HI, 4)   # chain 3
    u.enable_input(InpSel.SRC_1_HI, 5)   # chain 4
    u.enable_input(InpSel.CONST_2, 6)    # chain 5
    dp = u.datapath_config
    # b0: m0 = max(S0, C0)
    dp[0].enable_alu(AluOp.MAX, AluInp.PREV_ALU_OUT, AluInp.PREV_DELAY_0)
    dp[0].pass_through_delay(0, 1, 2, 3, 4, 5)
    # b1: m1 = max(S1, C1); chain1 <- m0
    dp[1].enable_alu(AluOp.MAX, AluInp.PREV_DELAY_1, AluInp.PREV_DELAY_2)
    dp[1].enable_delay_from_src(DelayInp.PREV_ALU_OUT, 1)
    dp[1].pass_through_delay(0, 2, 3, 4, 5)
    # b2: m1b = max(m1, C2); carry m0 on chain1
    dp[2].enable_alu(AluOp.MAX, AluInp.PREV_ALU_OUT, AluInp.PREV_DELAY_5)
    dp[2].pass_through_delay(0, 1, 2, 3, 4, 5)
    # b3: r_lo = m1b * m0
    dp[3].enable_alu(AluOp.MULTIPLY, AluInp.PREV_ALU_OUT, AluInp.PREV_DELAY_1)
    dp[3].pass_through_delay(0, 2, 3, 4, 5)
    # b4: m2 = max(S0_HI, C0); chain1 <- r_lo
    dp[4].enable_alu(AluOp.MAX, AluInp.PREV_DELAY_3, AluInp.PREV_DELAY_0)
    dp[4].enable_delay_from_src(DelayInp.PREV_ALU_OUT, 1)
    dp[4].pass_through_delay(2, 4, 5)
    # b5: m3 = max(S1_HI, C1); chain3 <- m2; carry r_lo
    dp[5].enable_alu(AluOp.MAX, AluInp.PREV_DELAY_4, AluInp.PREV_DELAY_2)
    dp[5].enable_delay_from_src(DelayInp.PREV_ALU_OUT, 3)
    dp[5].pass_through_delay(1, 5)
    # b6: m3b = max(m3, C2); carry r_lo, m2
    dp[6].enable_alu(AluOp.MAX, AluInp.PREV_ALU_OUT, AluInp.PREV_DELAY_5)
    dp[6].pass_through_delay(1, 3)
    # b7: r_hi = m3b * m2; carry r_lo
    dp[7].enable_alu(AluOp.MULTIPLY, AluInp.PREV_ALU_OUT, AluInp.PREV_DELAY_3)
    dp[7].pass_through_delay(1)
    u.enable_output(OutSel.DELAY_1, OutPath.WR0_LO)
    u.enable_output(OutSel.ALU_OUT, OutPath.WR0_HI)
    u.require_inp0 = 1
    u.require_inp1 = 1
    u.trigger = (Trigger.SRC_TENSOR_DONE, Trigger.NONE, Trigger.NONE)
    return u


def _register_gated_maxmul():
    """Register the fused out = max(in0,s0)*max(max(in1,s1),imm2) op."""
    import concourse.dve_ops as dve_ops
    from concourse.dve_ops import _COMPILE_CACHE, OPS, DveOp, has_src1
    from concourse.dve_spec import C0, C1, C2, Spec, Src0, Src1, lower, maxx
    from concourse.dve_uop import DveOpSpec

    NAME = "GATED_MAXMUL2XB_ANT"
    for op in OPS:
        if op.name == NAME:
            return op

    spec = Spec(
        body=maxx(Src0, C0) * maxx(maxx(Src1, C1), C2),
        reference=lambda in0, in1, s0, s1, imm2: np.maximum(in0, s0)
        * np.maximum(np.maximum(in1, s1), imm2),
    )
    op = DveOp(NAME, spec, subdim=False, uops_sha={})
    OPS.append(op)
    dve_ops.CUSTOM_DVE_SPECS[op.name] = op.spec
    opcode = dve_ops._CUSTOM_DVE_ROW_BASE + len(OPS) - 1
    assert opcode < 0x20
    dve_ops._SUB_OPCODE_FOR_NAME[op.name] = opcode
    s = DveOpSpec(
        name=op.name,
        opcode=opcode,
        uops=lower(spec, ver="v3"),
        uops_2x=[_build_2x_uop()] if PERF2X else None,
        perf_max=1 if PERF2X else 0,
        rd1_en=has_src1(spec),
    )
    op.uops_sha["v3"] = s.sha("v3")
    _COMPILE_CACHE[(op.name, "v3")] = s
    return op


def _build_program():
    import concourse.bacc as bacc
    import concourse.mybir as mybir
    import concourse.tile as tile
    from concourse.ap import AP

    dt = mybir.dt.float32
    f16 = mybir.dt.float16
    AF = mybir.ActivationFunctionType
    ALU = mybir.AluOpType

    gated_op = _register_gated_maxmul()

    nc = bacc.Bacc("TRN2", target_bir_lowering=False, debug=False, num_devices=NCORES)

    emb16 = nc.declare_dram_parameter("emb16", [C, N], f16, isOutput=False)
    # packed weights: [:, 0:EPC] = w1t, [:, EPC:2*EPC] = w2t
    wpack = nc.declare_dram_parameter("wpack", [C, 2 * EPC], f16, isOutput=False)
    # packed f32 smalls: [:, 0:1]=th5c1, [:, 1:2]=th5c2, [:, 2:2+EPC]=eyeT
    fpack = nc.declare_dram_parameter("fpack", [EPC, 2 + EPC], dt, isOutput=False)
    # packed fp16: [:, 0:P]=eye, row0 [0, P:2P]=ones
    epack = nc.declare_dram_parameter("epack", [P, 2 * P], f16, isOutput=False)
    out = nc.declare_dram_parameter("out", [EPC, N, N], f16, isOutput=True)

    with tile.TileContext(nc, pool_alloc_mode="queue") as tc:
        with (
            tc.tile_pool(name="const", bufs=1) as cpool,
            tc.tile_pool(name="rows", bufs=1) as rpool,
        ):
            # ACT-table warm-up: force both table loads before real work.
            sb_warm = cpool.tile([1, 8], dt)
            nc.vector.memset(sb_warm[:], 0.0)
            nc.scalar.activation(sb_warm[:], sb_warm[:], AF.Relu)
            nc.scalar.activation(sb_warm[:], sb_warm[:], AF.Sigmoid)

            sb_emb16 = cpool.tile([C, N], f16)
            nc.sync.dma_start(out=sb_emb16[:], in_=emb16[:])
            sb_w = cpool.tile([C, 2 * EPC], f16)
            nc.sync.dma_start(out=sb_w[:], in_=wpack[:])
            sb_f = cpool.tile([EPC, 2 + EPC], dt)
            nc.sync.dma_start(out=sb_f[:], in_=fpack[:])
            sb_e = cpool.tile([P, 2 * P], f16)
            nc.sync.dma_start(out=sb_e[:], in_=epack[:])
            sb_w1t = sb_w[:, 0:EPC]
            sb_w2t = sb_w[:, EPC:2 * EPC]
            sb_th5c1 = sb_f[:, 0:1]
            sb_th5c2 = sb_f[:, 1:2]
            sb_eyeT = sb_f[:, 2:2 + EPC]
            sb_eye16 = sb_e[:, 0:P]
            sb_ones16 = sb_e[0:1, P:2 * P]

            sb_vg16 = rpool.tile([EPC, 2 * N], f16)
            sb_dtrue = rpool.tile([EPC, N], dt)
            sb_vcol = rpool.tile([P, NB * EPC], dt)
            sb_gcol = rpool.tile([P, NB * EPC], dt)
            sb_dcol16 = rpool.tile([P, NB * EPC], f16)
            sb_flat = rpool.tile([1, EPC * 2 * N], f16)

            with (
                tc.tile_pool(name="jrepps", bufs=2, space="PSUM") as jps,
                tc.tile_pool(name="jrepsb", bufs=4) as jsb,
                tc.tile_pool(name="obuf", bufs=8) as opool,
            ):

                def emit_repl(ch):
                    """Replicate channel ch's v|g rows across 128 partitions.
                    Half-sized PSUM tiles (1 bank each) so this coexists
                    with the phase-1/phase-2 PSUM pools.  Channel 0's rows
                    already live at partition 0, so they feed the PE without
                    the partition-0 staging hop."""
                    fo = ch * 2 * N
                    rows = sb_vg16[0:1, :] if ch == 0 else sb_flat[0:1, fo:fo + 2 * N]
                    sb_vj = jsb.tile([P, N], f16, tag="sb_vj")
                    sb_gj = jsb.tile([P, N], f16, tag="sb_gj")
                    for h in range(2):
                        ps_vh = jps.tile([P, H], dt, tag="ps_h")
                        nc.tensor.matmul(
                            ps_vh[:],
                            lhsT=sb_ones16,
                            rhs=rows[0:1, h * H:(h + 1) * H],
                            start=True,
                            stop=True,
                        )
                        nc.scalar.copy(sb_vj[:, h * H:(h + 1) * H], ps_vh[:])
                    for h in range(2):
                        ps_gh = jps.tile([P, H], dt, tag="ps_h")
                        nc.tensor.matmul(
                            ps_gh[:],
                            lhsT=sb_ones16,
                            rhs=rows[0:1, N + h * H:N + (h + 1) * H],
                            start=True,
                            stop=True,
                        )
                        nc.scalar.copy(sb_gj[:, h * H:(h + 1) * H], ps_gh[:])
                    return sb_vj, sb_gj

                eye_ap = sb_eye16
                dcol_ap = sb_dcol16[:]

                def emit_patch(o_ap, blk0, nblk, ch):
                    patch_out = AP(
                        o_ap.tensor,
                        o_ap.offset + blk0 * (N + P),
                        [list(o_ap.ap[0]), [N + P, nblk], [1, P]],
                    )
                    patch_mask = AP(
                        eye_ap.tensor,
                        eye_ap.offset,
                        [list(eye_ap.ap[0]), [0, nblk], [1, P]],
                    ).bitcast(mybir.dt.int16)
                    patch_data = AP(
                        dcol_ap.tensor,
                        dcol_ap.offset + blk0 * EPC + ch,
                        [list(dcol_ap.ap[0]), [EPC, nblk], [0, P]],
                    )
                    nc.vector.copy_predicated(patch_out, patch_mask, patch_data)

                def emit_odma(o_ap, blk0, nblk, ch):
                    src = AP(
                        o_ap.tensor,
                        o_ap.offset + blk0 * N,
                        [list(o_ap.ap[0]), [N, nblk], [1, N]],
                    )
                    out_ap = out[:]
                    dst = AP(
                        out_ap.tensor,
                        ch * N * N + blk0 * P * N,
                        [[N, P], [P * N, nblk], [1, N]],
                    )
                    # Late channels issue from the scalar HWDGE queue so the
                    # burst of exit-DMA issues doesn't serialize on Sync.
                    eng = nc.scalar if ch >= EPC - 2 else nc.sync
                    eng.dma_start(out=dst, in_=src)

                def emit_mains(ch, sb_vj, sb_gj):
                    o = opool.tile([P, NB * N], f16, tag="o")
                    for r in range(NB):
                        ci = r * EPC + ch
                        inst = nc.vector._custom_dve(
                            gated_op,
                            out=o[:, r * N:(r + 1) * N],
                            in0=sb_vj[:],
                            in1=sb_gj[:],
                            s0=sb_vcol[:, ci:ci + 1],
                            s1=sb_gcol[:, ci:ci + 1],
                            imm2=0.5,
                        )
                        if PERF2X:
                            inst.ins.perf_max = 1
                    return o[:]

                with (
                    tc.tile_pool(name="ph1ps", bufs=1, space="PSUM") as p1ps,
                    tc.tile_pool(name="ph1sb", bufs=1) as p1sb,
                    tc.tile_pool(name="colps", bufs=2, space="PSUM") as cps,
                    tc.tile_pool(name="colsb", bufs=2) as csb,
                ):
                    ps_t1 = p1ps.tile([EPC, N], dt)
                    ps_t2 = p1ps.tile([EPC, N], dt)
                    for w, ps in ((sb_w1t, ps_t1), (sb_w2t, ps_t2)):
                        for h in range(2):
                            nc.tensor.matmul(
                                ps[:, h * H:(h + 1) * H],
                                lhsT=w,
                                rhs=sb_emb16[:, h * H:(h + 1) * H],
                                start=True,
                                stop=True,
                            )
                    nc.scalar.activation(sb_vg16[:, :N], ps_t1[:], AF.Relu, scale=2.0)
                    nc.scalar.activation(
                        sb_vg16[:, N:], ps_t2[:], AF.Sigmoid, scale=2.0
                    )
                    # Stage channels 1..7's v|g rows onto partition 0
                    # (channel 0 feeds the PE directly from partition 0).
                    for ch in range(1, EPC):
                        fo = ch * 2 * N
                        nc.sync.dma_start(
                            out=sb_flat[0:1, fo:fo + 2 * N],
                            in_=sb_vg16[ch:ch + 1, :],
                        )

                    # Phase-2 iteration 0 first: unblocks ch0's first DVE op.
                    def emit_col(r):
                        pv = cps.tile([P, EPC], dt, tag="pc")
                        nc.tensor.matmul(
                            pv[:],
                            lhsT=sb_emb16[:, r * P:(r + 1) * P],
                            rhs=sb_w1t,
                            start=True,
                            stop=True,
                        )
                        nc.scalar.activation(
                            sb_vcol[:, r * EPC:(r + 1) * EPC], pv[:],
                            AF.Relu, scale=2.0,
                        )
                        pg = cps.tile([P, EPC], dt, tag="pc")
                        nc.tensor.matmul(
                            pg[:],
                            lhsT=sb_emb16[:, r * P:(r + 1) * P],
                            rhs=sb_w2t,
                            start=True,
                            stop=True,
                        )
                        nc.scalar.activation(
                            sb_gcol[:, r * EPC:(r + 1) * EPC], pg[:],
                            AF.Sigmoid, scale=2.0,
                        )

                    emit_col(0)
                    repl = {0: emit_repl(0)}
                    for r in range(1, NB):
                        emit_col(r)
                    repl[1] = emit_repl(1)

                    # Diagonal-value ACTs (scalar queue, ahead of the dcol
                    # copies but after the vj0 copies emitted above).
                    sb_d1 = p1sb.tile([EPC, N], dt)
                    nc.scalar.activation(
                        sb_d1[:], ps_t1[:], AF.Relu, bias=sb_th5c1, scale=2.0
                    )
                    sb_d2 = p1sb.tile([EPC, N], dt)
                    nc.scalar.activation(
                        sb_d2[:], ps_t2[:], AF.Sigmoid, bias=sb_th5c2, scale=2.0
                    )

                    # Channel 0 main ops go into the (in-order) vector queue
                    # BEFORE the dtrue op, so the first custom op fires as
                    # soon as vj/vcol exist; the dtrue combine then slots in
                    # behind them without gating anything early.
                    o0 = emit_mains(0, *repl[0])
                    nc.vector.scalar_tensor_tensor(
                        sb_dtrue[:], sb_d2[:], 0.5, sb_d1[:], ALU.max, ALU.mult
                    )
                    for r in range(NB):
                        pt_c = cps.tile([P, EPC], dt, tag="pc")
                        nc.tensor.transpose(
                            pt_c[:], sb_dtrue[:, r * P:(r + 1) * P], sb_eyeT
                        )
                        nc.scalar.copy(
                            sb_dcol16[:, r * EPC:(r + 1) * EPC], pt_c[:]
                        )

                # Channel 0 was computed inside the phase-1 scope; ship it.
                emit_patch(o0, 0, NB, 0)
                emit_odma(o0, 0, NB, 0)
                for ch in range(1, EPC):
                    sb_vj, sb_gj = repl[ch] if ch in repl else emit_repl(ch)
                    # Later channels split their exit DMA so the tail after
                    # the last DVE op shrinks.
                    splits = 4 if ch >= EPC - 2 else (2 if ch == EPC - 3 else 1)
                    step = NB // splits
                    o = opool.tile([P, NB * N], f16, tag="o")
                    o_ap = o[:]
                    for r in range(NB):
                        ci = r * EPC + ch
                        inst = nc.vector._custom_dve(
                            gated_op,
                            out=o[:, r * N:(r + 1) * N],
                            in0=sb_vj[:],
                            in1=sb_gj[:],
                            s0=sb_vcol[:, ci:ci + 1],
                            s1=sb_gcol[:, ci:ci + 1],
                            imm2=0.5,
                        )
                        if PERF2X:
                            inst.ins.perf_max = 1
                        if (r + 1) % step == 0:
                            emit_patch(o_ap, r + 1 - step, step, ch)
                            emit_odma(o_ap, r + 1 - step, step, ch)

    nc.compile()
    return nc


def _get_program():
    if "nc" not in _CACHE:
        _CACHE["nc"] = _build_program()
    return _CACHE["nc"]


def kernel(**inputs):
    _ensure_hook_shim()
    from concourse.bass_utils import run_bass_kernel_spmd

    emb = np.asarray(inputs["emb"], dtype=np.float32)
    th12_1 = np.asarray(inputs["th12_1"], dtype=np.float32)
    th12_2 = np.asarray(inputs["th12_2"], dtype=np.float32)
    th5_1 = np.asarray(inputs["th5_1"], dtype=np.float32)
    th5_2 = np.asarray(inputs["th5_2"], dtype=np.float32)

    epack = np.zeros((P, 2 * P), dtype=np.float16)
    epack[:, :P] = np.eye(P, dtype=np.float16)
    epack[0, P:] = 1.0

    in_maps = []
    for k in range(NCORES):
        b = k // (NCORES // B)
        e0 = (k % (NCORES // B)) * EPC
        wpack = np.empty((C, 2 * EPC), dtype=np.float16)
        wpack[:, :EPC] = th12_1[e0:e0 + EPC].T
        wpack[:, EPC:] = th12_2[e0:e0 + EPC].T
        fpack = np.empty((EPC, 2 + EPC), dtype=np.float32)
        fpack[:, 0] = th5_1[e0:e0 + EPC]
        fpack[:, 1] = th5_2[e0:e0 + EPC]
        fpack[:, 2:] = np.eye(EPC, dtype=np.float32)
        in_maps.append(
            {
                "emb16": np.ascontiguousarray(emb[b].astype(np.float16)),
                "wpack": wpack,
                "fpack": fpack,
                "epack": epack,
            }
        )

    nc = _get_program()
    res = run_bass_kernel_spmd(nc, in_maps, core_ids=list(range(NCORES)))
    _CACHE["last_result"] = res

    out = np.empty((B, E, N, N), dtype=np.float32)
    for k in range(NCORES):
        b = k // (NCORES // B)
        e0 = (k % (NCORES // B)) * EPC
        out[b, e0:e0 + EPC] = res.results[k]["out"].astype(np.float32)
    return out


# revision 30
# speedup vs baseline: 1.0491x; 1.0491x over previous
"""Trainium2 Bass/Tile kernel for the GatedNode2Edge op.

Computes, for emb (B,C,N), th12_* (E,C), th5_* (E,):
    t_k  = th12_k @ emb[b]                      (E,N)
    m_k  = max(t_k[:,i], t_k[:,j]) pairwise     (E,N,N)
    adj  = relu(2*m_1 + th5_1*I)
    gate = sigmoid(relu(2*m_2 + th5_2*I))
    out  = adj * gate                           (B,E,N,N)

Sharding: the 64 (b,e) channels are split 8-per-core across 8 NeuronCores.

Math restructuring (off-diagonal):
    relu(2*max(a,b)) = max(2*relu(a), 2*relu(b))           (relu monotone)
    sigmoid(max(x,y)) = max(sigmoid(x), sigmoid(y))        (sigmoid monotone)
    sigmoid(2*relu(t)) = max(sigmoid(2t), 1/2)             (relu fold)
so with row vectors v = 2*relu(t1), g = sigmoid(2*t2):
    out[i,j] = max(v_i, v_j) * max(g_i, g_j, 1/2)
one fused custom-DVE op per [128, N] output tile:
    out = maxx(Src0, C0) * maxx(Src1, C1)
with Src0 = v broadcast across partitions (PE outer-product), C0 = v column
slice (per-partition scalar), Src1 = sigmoid(2t2) broadcast, and C1 =
max(g_i, 1/2) column slice (the relu fold rides in the scalar).

Performance structure vs the f32 baseline:
  - All pairwise tiles are fp16; custom DVE op runs a hand-authored
    2X_1PORT uop program (2 outputs/cycle); fp16 matmuls (single pass).
  - Replication PSUM is allocated in 1-bank halves so it coexists with
    the phase-1 PSUM: channel 0/1 replication overlaps the phase-1 tail
    instead of stalling on a PSUM-bank WAR against phase 2.
  - Emission order hand-interleaved so the first custom-DVE op fires as
    early as possible; ACT tables are pre-warmed by dummy activations.
  - True diagonal patched with ONE strided copy_predicated per channel;
    output leaves per channel as one 2 MB DMA (last channel split in two
    to shorten the tail).
"""

import sys
import types

import numpy as np

B, C, N, E = 2, 64, 1024, 32
NCORES = 8
EPC = B * E // NCORES  # 8 channels per core
P = 128
NB = N // P  # 8 row blocks
H = N // 2  # matmul moving free-dim limit is 512

PERF2X = True  # set False to fall back to 1x custom-DVE mode

_CACHE = {}


def _ensure_hook_shim():
    """Make trace=True safe even when antenv.axon_hooks is absent."""
    try:
        import antenv.axon_hooks  # noqa: F401
    except ImportError:
        mod = types.ModuleType("antenv.axon_hooks")
        mod.get_axon_ntff_profile_hook = lambda: None
        mod.set_axon_ntff_profile_hook = lambda h: None
        sys.modules["antenv.axon_hooks"] = mod


def _build_2x_uop():
    """2X_1PORT program for out = max(Src0,C0)*max(max(Src1,C1),C2).

    Per cycle the engine delivers packed fp16 pairs: (SRC_0, SRC_0_HI) from
    rd0 and (SRC_1, SRC_1_HI) from rd1.  All eight ALU blocks compute (four
    ops per result, two results per cycle); the even (lo) result rides delay
    chain 1 to the writeback, the odd (hi) result leaves via block-7 ALU out.
    """
    from concourse.dve_uop import (
        AluInp,
        AluOp,
        DelayInp,
        InpSel,
        OutPath,
        OutSel,
        Trigger,
        UopConfig,
    )

    u = UopConfig()
    # lane 0 feeds block 0's ALU directly; lanes 1..6 feed delay chains 0..5
    u.enable_input(InpSel.SRC_0, 0)      # direct ALU input at b0
    u.enable_input(InpSel.CONST_0, 1)    # chain 0
    u.enable_input(InpSel.SRC_1, 2)      # chain 1
    u.enable_input(InpSel.CONST_1, 3)    # chain 2
    u.enable_input(InpSel.SRC_0_HI, 4)   # chain 3
    u.enable_input(InpSel.SRC_1_HI, 5)   # chain 4
    u.enable_input(InpSel.CONST_2, 6)    # chain 5
    dp = u.datapath_config
    # b0: m0 = max(S0, C0)
    dp[0].enable_alu(AluOp.MAX, AluInp.PREV_ALU_OUT, AluInp.PREV_DELAY_0)
    dp[0].pass_through_delay(0, 1, 2, 3, 4, 5)
    # b1: m1 = max(S1, C1); chain1 <- m0
    dp[1].enable_alu(AluOp.MAX, AluInp.PREV_DELAY_1, AluInp.PREV_DELAY_2)
    dp[1].enable_delay_from_src(DelayInp.PREV_ALU_OUT, 1)
    dp[1].pass_through_delay(0, 2, 3, 4, 5)
    # b2: m1b = max(m1, C2); carry m0 on chain1
    dp[2].enable_alu(AluOp.MAX, AluInp.PREV_ALU_OUT, AluInp.PREV_DELAY_5)
    dp[2].pass_through_delay(0, 1, 2, 3, 4, 5)
    # b3: r_lo = m1b * m0
    dp[3].enable_alu(AluOp.MULTIPLY, AluInp.PREV_ALU_OUT, AluInp.PREV_DELAY_1)
    dp[3].pass_through_delay(0, 2, 3, 4, 5)
    # b4: m2 = max(S0_HI, C0); chain1 <- r_lo
    dp[4].enable_alu(AluOp.MAX, AluInp.PREV_DELAY_3, AluInp.PREV_DELAY_0)
    dp[4].enable_delay_from_src(DelayInp.PREV_ALU_OUT, 1)
    dp[4].pass_through_delay(2, 4, 5)
    # b5: m3 = max(S1_HI, C1); chain3 <- m2; carry r_lo
    dp[5].enable_alu(AluOp.MAX, AluInp.PREV_DELAY_4, AluInp.PREV_DELAY_2)
    dp[5].enable_delay_from_src(DelayInp.PREV_ALU_OUT, 3)
    dp[5].pass_through_delay(1, 5)
    # b6: m3b = max(m3, C2); carry r_lo, m2
    dp[6].enable_alu(AluOp.MAX, AluInp.PREV_ALU_OUT, AluInp.PREV_DELAY_5)
    dp[6].pass_through_delay(1, 3)
    # b7: r_hi = m3b * m2; carry r_lo
    dp[7].enable_alu(AluOp.MULTIPLY, AluInp.PREV_ALU_OUT, AluInp.PREV_DELAY_3)
    dp[7].pass_through_delay(1)
    u.enable_output(OutSel.DELAY_1, OutPath.WR0_LO)
    u.enable_output(OutSel.ALU_OUT, OutPath.WR0_HI)
    u.require_inp0 = 1
    u.require_inp1 = 1
    u.trigger = (Trigger.SRC_TENSOR_DONE, Trigger.NONE, Trigger.NONE)
    return u


def _register_gated_maxmul():
    """Register the fused out = max(in0,s0)*max(max(in1,s1),imm2) op."""
    import concourse.dve_ops as dve_ops
    from concourse.dve_ops import _COMPILE_CACHE, OPS, DveOp, has_src1
    from concourse.dve_spec import C0, C1, C2, Spec, Src0, Src1, lower, maxx
    from concourse.dve_uop import DveOpSpec

    NAME = "GATED_MAXMUL2XB_ANT"
    for op in OPS:
        if op.name == NAME:
            return op

    spec = Spec(
        body=maxx(Src0, C0) * maxx(maxx(Src1, C1), C2),
        reference=lambda in0, in1, s0, s1, imm2: np.maximum(in0, s0)
        * np.maximum(np.maximum(in1, s1), imm2),
    )
    op = DveOp(NAME, spec, subdim=False, uops_sha={})
    OPS.append(op)
    dve_ops.CUSTOM_DVE_SPECS[op.name] = op.spec
    opcode = dve_ops._CUSTOM_DVE_ROW_BASE + len(OPS) - 1
    assert opcode < 0x20
    dve_ops._SUB_OPCODE_FOR_NAME[op.name] = opcode
    s = DveOpSpec(
        name=op.name,
        opcode=opcode,
        uops=lower(spec, ver="v3"),
        uops_2x=[_build_2x_uop()] if PERF2X else None,
        perf_max=1 if PERF2X else 0,
        rd1_en=has_src1(spec),
    )
    op.uops_sha["v3"] = s.sha("v3")
    _COMPILE_CACHE[(op.name, "v3")] = s
    return op


def _build_program():
    import concourse.bacc as bacc
    import concourse.mybir as mybir
    import concourse.tile as tile
    from concourse.ap import AP

    dt = mybir.dt.float32
    f16 = mybir.dt.float16
    AF = mybir.ActivationFunctionType
    ALU = mybir.AluOpType

    gated_op = _register_gated_maxmul()

    nc = bacc.Bacc("TRN2", target_bir_lowering=False, debug=False, num_devices=NCORES)

    emb16 = nc.declare_dram_parameter("emb16", [C, N], f16, isOutput=False)
    # packed weights: [:, 0:EPC] = w1t, [:, EPC:2*EPC] = w2t
    wpack = nc.declare_dram_parameter("wpack", [C, 2 * EPC], f16, isOutput=False)
    # packed f32 smalls: [:, 0:1]=th5c1, [:, 1:2]=th5c2, [:, 2:2+EPC]=eyeT
    fpack = nc.declare_dram_parameter("fpack", [EPC, 2 + EPC], dt, isOutput=False)
    # packed fp16: [:, 0:P]=eye, row0 [0, P:2P]=ones
    epack = nc.declare_dram_parameter("epack", [P, 2 * P], f16, isOutput=False)
    out = nc.declare_dram_parameter("out", [EPC, N, N], f16, isOutput=True)

    with tile.TileContext(nc, pool_alloc_mode="queue") as tc:
        with (
            tc.tile_pool(name="const", bufs=1) as cpool,
            tc.tile_pool(name="rows", bufs=1) as rpool,
        ):
            # ACT-table warm-up: force both table loads before real work.
            sb_warm = cpool.tile([1, 8], dt)
            nc.gpsimd.memset(sb_warm[:], 0.0)
            nc.scalar.activation(sb_warm[:], sb_warm[:], AF.Relu)
            nc.scalar.activation(sb_warm[:], sb_warm[:], AF.Sigmoid)

            # epack (carrying ones16, which gates the first replication
            # matmuls) and the weights issue before the big emb transfer.
            sb_e = cpool.tile([P, 2 * P], f16)
            nc.sync.dma_start(out=sb_e[:], in_=epack[:])
            sb_w = cpool.tile([C, 2 * EPC], f16)
            nc.sync.dma_start(out=sb_w[:], in_=wpack[:])
            sb_emb16 = cpool.tile([C, N], f16)
            nc.sync.dma_start(out=sb_emb16[:], in_=emb16[:])
            sb_f = cpool.tile([EPC, 2 + EPC], dt)
            nc.sync.dma_start(out=sb_f[:], in_=fpack[:])
            sb_w1t = sb_w[:, 0:EPC]
            sb_w2t = sb_w[:, EPC:2 * EPC]
            sb_th5c1 = sb_f[:, 0:1]
            sb_th5c2 = sb_f[:, 1:2]
            sb_eyeT = sb_f[:, 2:2 + EPC]
            sb_eye16 = sb_e[:, 0:P]
            sb_ones16 = sb_e[0:1, P:2 * P]

            sb_vg16 = rpool.tile([EPC, 2 * N], f16)
            sb_dtrue = rpool.tile([EPC, N], dt)
            sb_vcol = rpool.tile([P, NB * EPC], dt)
            sb_gcol = rpool.tile([P, NB * EPC], dt)
            sb_dcol16 = rpool.tile([P, NB * EPC], f16)
            sb_flat = rpool.tile([1, EPC * 2 * N], f16)

            with (
                tc.tile_pool(name="jrepps", bufs=2, space="PSUM") as jps,
                tc.tile_pool(name="jrepsb", bufs=4) as jsb,
                tc.tile_pool(name="obuf", bufs=8) as opool,
            ):

                def emit_repl(ch):
                    """Replicate channel ch's v|g rows across 128 partitions.
                    Half-sized PSUM tiles (1 bank each) so this coexists
                    with the phase-1/phase-2 PSUM pools.  Channel 0's rows
                    already live at partition 0, so they feed the PE without
                    the partition-0 staging hop."""
                    fo = ch * 2 * N
                    rows = sb_vg16[0:1, :] if ch == 0 else sb_flat[0:1, fo:fo + 2 * N]
                    sb_vj = jsb.tile([P, N], f16, tag="sb_vj")
                    sb_gj = jsb.tile([P, N], f16, tag="sb_gj")
                    for h in range(2):
                        ps_vh = jps.tile([P, H], dt, tag="ps_h")
                        nc.tensor.matmul(
                            ps_vh[:],
                            lhsT=sb_ones16,
                            rhs=rows[0:1, h * H:(h + 1) * H],
                            start=True,
                            stop=True,
                        )
                        nc.scalar.copy(sb_vj[:, h * H:(h + 1) * H], ps_vh[:])
                    for h in range(2):
                        ps_gh = jps.tile([P, H], dt, tag="ps_h")
                        nc.tensor.matmul(
                            ps_gh[:],
                            lhsT=sb_ones16,
                            rhs=rows[0:1, N + h * H:N + (h + 1) * H],
                            start=True,
                            stop=True,
                        )
                        nc.scalar.copy(sb_gj[:, h * H:(h + 1) * H], ps_gh[:])
                    return sb_vj, sb_gj

                eye_ap = sb_eye16
                dcol_ap = sb_dcol16[:]

                def emit_patch(o_ap, blk0, nblk, ch):
                    patch_out = AP(
                        o_ap.tensor,
                        o_ap.offset + blk0 * (N + P),
                        [list(o_ap.ap[0]), [N + P, nblk], [1, P]],
                    )
                    patch_mask = AP(
                        eye_ap.tensor,
                        eye_ap.offset,
                        [list(eye_ap.ap[0]), [0, nblk], [1, P]],
                    ).bitcast(mybir.dt.int16)
                    patch_data = AP(
                        dcol_ap.tensor,
                        dcol_ap.offset + blk0 * EPC + ch,
                        [list(dcol_ap.ap[0]), [EPC, nblk], [0, P]],
                    )
                    nc.vector.copy_predicated(patch_out, patch_mask, patch_data)

                def emit_odma(o_ap, blk0, nblk, ch):
                    src = AP(
                        o_ap.tensor,
                        o_ap.offset + blk0 * N,
                        [list(o_ap.ap[0]), [N, nblk], [1, N]],
                    )
                    out_ap = out[:]
                    dst = AP(
                        out_ap.tensor,
                        ch * N * N + blk0 * P * N,
                        [[N, P], [P * N, nblk], [1, N]],
                    )
                    nc.sync.dma_start(out=dst, in_=src)

                def emit_mains(ch, sb_vj, sb_gj):
                    o = opool.tile([P, NB * N], f16, tag="o")
                    for r in range(NB):
                        ci = r * EPC + ch
                        inst = nc.vector._custom_dve(
                            gated_op,
                            out=o[:, r * N:(r + 1) * N],
                            in0=sb_vj[:],
                            in1=sb_gj[:],
                            s0=sb_vcol[:, ci:ci + 1],
                            s1=sb_gcol[:, ci:ci + 1],
                            imm2=0.5,
                        )
                        if PERF2X:
                            inst.ins.perf_max = 1
                    return o[:]

                with (
                    tc.tile_pool(name="ph1ps", bufs=1, space="PSUM") as p1ps,
                    tc.tile_pool(name="ph1sb", bufs=1) as p1sb,
                    tc.tile_pool(name="colps", bufs=2, space="PSUM") as cps,
                    tc.tile_pool(name="colsb", bufs=2) as csb,
                ):
                    ps_t1 = p1ps.tile([EPC, N], dt)
                    ps_t2 = p1ps.tile([EPC, N], dt)
                    for w, ps in ((sb_w1t, ps_t1), (sb_w2t, ps_t2)):
                        for h in range(2):
                            nc.tensor.matmul(
                                ps[:, h * H:(h + 1) * H],
                                lhsT=w,
                                rhs=sb_emb16[:, h * H:(h + 1) * H],
                                start=True,
                                stop=True,
                            )
                    nc.scalar.activation(sb_vg16[:, :N], ps_t1[:], AF.Relu, scale=2.0)
                    nc.scalar.activation(
                        sb_vg16[:, N:], ps_t2[:], AF.Sigmoid, scale=2.0
                    )
                    # Stage channels 1..7's v|g rows onto partition 0
                    # (channel 0 feeds the PE directly from partition 0).
                    for ch in range(1, EPC):
                        fo = ch * 2 * N
                        nc.sync.dma_start(
                            out=sb_flat[0:1, fo:fo + 2 * N],
                            in_=sb_vg16[ch:ch + 1, :],
                        )

                    # Phase-2 iteration 0 first: unblocks ch0's first DVE op.
                    def emit_col(r):
                        pv = cps.tile([P, EPC], dt, tag="pc")
                        nc.tensor.matmul(
                            pv[:],
                            lhsT=sb_emb16[:, r * P:(r + 1) * P],
                            rhs=sb_w1t,
                            start=True,
                            stop=True,
                        )
                        nc.scalar.activation(
                            sb_vcol[:, r * EPC:(r + 1) * EPC], pv[:],
                            AF.Relu, scale=2.0,
                        )
                        pg = cps.tile([P, EPC], dt, tag="pc")
                        nc.tensor.matmul(
                            pg[:],
                            lhsT=sb_emb16[:, r * P:(r + 1) * P],
                            rhs=sb_w2t,
                            start=True,
                            stop=True,
                        )
                        nc.scalar.activation(
                            sb_gcol[:, r * EPC:(r + 1) * EPC], pg[:],
                            AF.Sigmoid, scale=2.0,
                        )

                    emit_col(0)
                    repl = {0: emit_repl(0)}
                    for r in range(1, NB):
                        emit_col(r)
                    repl[1] = emit_repl(1)

                    # Diagonal-value ACTs (scalar queue, ahead of the dcol
                    # copies but after the vj0 copies emitted above).
                    sb_d1 = p1sb.tile([EPC, N], dt)
                    nc.scalar.activation(
                        sb_d1[:], ps_t1[:], AF.Relu, bias=sb_th5c1, scale=2.0
                    )
                    sb_d2 = p1sb.tile([EPC, N], dt)
                    nc.scalar.activation(
                        sb_d2[:], ps_t2[:], AF.Sigmoid, bias=sb_th5c2, scale=2.0
                    )

                    # Channel 0 main ops go into the (in-order) vector queue
                    # BEFORE the dtrue op, so the first custom op fires as
                    # soon as vj/vcol exist; the dtrue combine then slots in
                    # behind them without gating anything early.
                    o0 = emit_mains(0, *repl[0])
                    nc.vector.scalar_tensor_tensor(
                        sb_dtrue[:], sb_d2[:], 0.5, sb_d1[:], ALU.max, ALU.mult
                    )
                    for r in range(NB):
                        pt_c = cps.tile([P, EPC], dt, tag="pc")
                        nc.tensor.transpose(
                            pt_c[:], sb_dtrue[:, r * P:(r + 1) * P], sb_eyeT
                        )
                        nc.scalar.copy(
                            sb_dcol16[:, r * EPC:(r + 1) * EPC], pt_c[:]
                        )

                # Channel 0 was computed inside the phase-1 scope; ship it.
                emit_patch(o0, 0, NB, 0)
                emit_odma(o0, 0, NB, 0)
                for ch in range(1, EPC):
                    sb_vj, sb_gj = repl[ch] if ch in repl else emit_repl(ch)
                    # Later channels split their exit DMA so the tail after
                    # the last DVE op shrinks.
                    splits = 4 if ch == EPC - 1 else (2 if ch == EPC - 2 else 1)
                    step = NB // splits
                    o = opool.tile([P, NB * N], f16, tag="o")
                    o_ap = o[:]
                    for r in range(NB):
                        ci = r * EPC + ch
                        inst = nc.vector._custom_dve(
                            gated_op,
                            out=o[:, r * N:(r + 1) * N],
                            in0=sb_vj[:],
                            in1=sb_gj[:],
                            s0=sb_vcol[:, ci:ci + 1],
                            s1=sb_gcol[:, ci:ci + 1],
                            imm2=0.5,
                        )
                        if PERF2X:
                            inst.ins.perf_max = 1
                        if (r + 1) % step == 0:
                            emit_patch(o_ap, r + 1 - step, step, ch)
                            emit_odma(o_ap, r + 1 - step, step, ch)

    nc.compile()
    return nc


def _get_program():
    if "nc" not in _CACHE:
        _CACHE["nc"] = _build_program()
    return _CACHE["nc"]


def kernel(**inputs):
    _ensure_hook_shim()
    from concourse.bass_utils import run_bass_kernel_spmd

    emb = np.asarray(inputs["emb"], dtype=np.float32)
    th12_1 = np.asarray(inputs["th12_1"], dtype=np.float32)
    th12_2 = np.asarray(inputs["th12_2"], dtype=np.float32)
    th5_1 = np.asarray(inputs["th5_1"], dtype=np.float32)
    th5_2 = np.asarray(inputs["th5_2"], dtype=np.float32)

    epack = np.zeros((P, 2 * P), dtype=np.float16)
    epack[:, :P] = np.eye(P, dtype=np.float16)
    epack[0, P:] = 1.0

    in_maps = []
    for k in range(NCORES):
        b = k // (NCORES // B)
        e0 = (k % (NCORES // B)) * EPC
        wpack = np.empty((C, 2 * EPC), dtype=np.float16)
        wpack[:, :EPC] = th12_1[e0:e0 + EPC].T
        wpack[:, EPC:] = th12_2[e0:e0 + EPC].T
        fpack = np.empty((EPC, 2 + EPC), dtype=np.float32)
        fpack[:, 0] = th5_1[e0:e0 + EPC]
        fpack[:, 1] = th5_2[e0:e0 + EPC]
        fpack[:, 2:] = np.eye(EPC, dtype=np.float32)
        in_maps.append(
            {
                "emb16": np.ascontiguousarray(emb[b].astype(np.float16)),
                "wpack": wpack,
                "fpack": fpack,
                "epack": epack,
            }
        )

    nc = _get_program()
    res = run_bass_kernel_spmd(nc, in_maps, core_ids=list(range(NCORES)))
    _CACHE["last_result"] = res

    out = np.empty((B, E, N, N), dtype=np.float32)
    for k in range(NCORES):
        b = k // (NCORES // B)
        e0 = (k % (NCORES // B)) * EPC
        out[b, e0:e0 + EPC] = res.results[k]["out"].astype(np.float32)
    return out
